# revision 10
# baseline (speedup 1.0000x reference)
"""Trainium2 Bass kernel for nn_Attention2d (sparse_attention) — v5.

Math (per reference):
  x: (2, 128, 64, 64); T = 4096 tokens; 4 heads x 32 channels.
  qkv 1x1-conv -> per-head attention over T -> 1x1-conv out proj -> residual.

Sharding: one (batch, head) pair per core (8 cores). Each core computes its
head's attention fully on-chip and returns the per-head partial of the
output projection (bf16); the host sums the 4 head partials per batch and
adds the residual + biases (exact, fp32).

v5 design (vs v3 baseline at ~163us):
  - QK scores: bf16 matmuls (K=32 contraction, 4-band replicated q/k with
    tile-position rotation) — 1 col/cycle is the PE floor here.
  - PV: fp8 DoubleRow matmuls with the pair dim = s-chunk parity: each
    matmul contracts 256 s-tokens ([128 parts, 2 chunks, 512] plane-split
    layout), HALVING the PV column count vs bf16. vT (e4m3) carries
    32 v rows + 32 ones rows per plane so the softmax denominator
    accumulates in the same matmul.
  - softmax exp: p is e5m2 with global factor 1. ScalarE supers use
    activation-Exp directly to e5m2; VectorE supers use a Schraudolph
    bit-trick (one tensor_scalar mult+add writing int8 bits that ARE the
    e5m2 of 2^(y*log2e)). Supers (2 s-chunks = 1024 fp32 PSUM cols,
    pair-aligned) alternate between the engines ~9:7.
  - epilogue per t-block: ONE ScalarE copy drains the PV accumulator
    (frees the PSUM bank before the next t-block's first PV), a DMA
    re-bases the denominator rows to partitions 0-31, DVE reciprocal +
    multiply normalize, bf16 out-projection, ScalarE copy, bf16 DMA out
    (host accumulates partials in fp32).
  - x arrives in 8 chunked DMAs so the first projection starts early.
"""

import numpy as np
import ml_dtypes

B, C, Hh, Ww = 2, 128, 64, 64
T = Hh * Ww          # 4096
NH, CH = 4, 32
N_CORES = 8
NCH = 32             # s-chunks (of 128 tokens) per t-block
NTB = 8              # t-blocks (of 512 tokens)
NSUP = NCH // 2      # 16 supers (2 chunks each) per t-block

SCALE2 = float(1.0 / np.sqrt(CH))
# e5m2 Schraudolph: bits = round(raw*C1 + C2); int8 bits viewed as e5m2
# == 2^((bits-60)/4) ~= exp(raw*SCALE2) with global factor 1.
C1 = SCALE2 * float(np.log2(np.e)) * 4.0
C2 = 60.0

# engine for each super (S=scalar-activation, D=vector-schraudolph), 9:7
ENG_PATTERN = ['S', 'D', 'S', 'D', 'S', 'D', 'S', 'S', 'D', 'S', 'D',
               'S', 'D', 'S', 'S', 'D']

_cache = {}


def _build_nc():
    import concourse.tile as tile
    from concourse import bacc, mybir

    BF16 = mybir.dt.bfloat16
    F32 = mybir.dt.float32
    I8 = mybir.dt.int8
    F8E4 = mybir.dt.float8e4
    F8E5 = mybir.dt.float8e5
    Exp = mybir.ActivationFunctionType.Exp
    Ident = mybir.ActivationFunctionType.Identity
    MULT = mybir.AluOpType.mult
    ADD = mybir.AluOpType.add
    DR = mybir.MatmulPerfMode.DoubleRow

    nc = bacc.Bacc("TRN2", target_bir_lowering=False, debug=False,
                   num_devices=N_CORES)

    x_in = nc.dram_tensor("x", [128, T], BF16, kind="ExternalInput")
    # packed weights: wqT(4-rep) | wkT(4-rep) | wvT | wpT  ([128, 416])
    w_in = nc.dram_tensor("wpack", [128, 416], BF16, kind="ExternalInput")
    # per-partition biases (4-replicated): bq | bk
    b_in = nc.dram_tensor("bpack", [128, 2], F32, kind="ExternalInput")
    out_t = nc.dram_tensor("out", [128, T], BF16, kind="ExternalOutput")

    with tile.TileContext(nc) as tc:
        with (
            tc.tile_pool(name="const", bufs=1) as cpool,
            tc.tile_pool(name="work", bufs=2) as wpool,
            tc.tile_pool(name="psum", bufs=1, space="PSUM") as pspool,
        ):
            # ---- input DMAs (x in 8 chunks so compute starts early) ----
            x_c = []
            for j in range(NTB):
                xt = cpool.tile([128, 512], BF16, tag=f"x_{j}",
                                name=f"x_{j}")
                nc.sync.dma_start(xt[:], x_in[:, j * 512:(j + 1) * 512])
                x_c.append(xt)
            w_sb = cpool.tile([128, 416], BF16)
            nc.sync.dma_start(w_sb[:], w_in[:])
            b_sb = cpool.tile([128, 2], F32)
            nc.sync.dma_start(b_sb[:], b_in[:])
            wq_sb = w_sb[:, 0:128]
            wk_sb = w_sb[:, 128:256]
            wv_sb = w_sb[:, 256:288]
            wpT_sb = w_sb[0:32, 288:416]
            bq_sb = b_sb[:, 0:1]
            bk_sb = b_sb[:, 1:2]

            q_sb = cpool.tile([128, T], BF16)
            k_sb = cpool.tile([128, T], BF16)
            # vT pair-blocks: block m: [2 planes x (32 v + 32 ones)] e4m3
            vT_sb = cpool.tile([128, 64 * NCH], F8E4)
            nc.gpsimd.memset(vT_sb[:], 1.0)

            # ---- helpers ----
            def pp_tile(nm):
                return pspool.tile([128, 512], F32, tag="pp", bufs=1,
                                   name=nm)

            def emit_proj(dst, wsb, bsb, j, nm, eng):
                """one 512-col t-chunk of the q/k projection (4-band
                replicated), bias added in the PSUM->bf16 copy."""
                ps = pp_tile(nm)
                nc.tensor.matmul(ps[:], wsb, x_c[j][:],
                                 start=True, stop=True)
                dstsl = dst[:, 512 * j:512 * (j + 1)]
                if eng == 'S':
                    nc.scalar.activation(dstsl, ps[:], Ident, bias=bsb,
                                         scale=1.0)
                else:
                    nc.vector.tensor_scalar_add(dstsl, ps[:], bsb)

            def emit_v_group(g):
                """v^T projection for s-chunks 4g..4g+3 (x chunk g) into
                vT pair-blocks 2g, 2g+1 (plane-split, e4m3)."""
                ps = pp_tile(f"pp_v{g}")
                for cc in range(4):
                    nc.tensor.matmul(
                        ps[:, 32 * cc:32 * (cc + 1)],
                        x_c[g][:, 128 * cc:128 * (cc + 1)],
                        wv_sb, start=True, stop=True)
                src = ps[:, 0:128].rearrange("p (c m) -> p c m", m=32)
                dst = vT_sb[:, 256 * g:256 * (g + 1)].rearrange(
                    "p (c m) -> p c m", m=64)[:, :, 0:32]
                nc.vector.tensor_copy(dst, src)

            # ---- prologue: minimum for t-block 0 ----
            emit_proj(k_sb, wk_sb, bk_sb, 0, "pp_k0", 'S')
            emit_proj(q_sb, wq_sb, bq_sb, 0, "pp_q0", 'D')
            emit_v_group(0)

            # ---- attention, software-pipelined over supers ----
            st_tiles = {}
            p_tiles = {}
            pv_tile = pspool.tile([64, 512], F32, tag="pv", bufs=1,
                                  name="pv")
            deferred = {}

            def defer(u, fn):
                deferred.setdefault(u, []).append(fn)

            def emit_qk(u):
                tb, su = divmod(u, NSUP)
                st = pspool.tile([128, 1024], F32, tag="st", bufs=3,
                                 name=f"st_{u}")
                st_tiles[u] = st
                tsl = slice(512 * tb, 512 * (tb + 1))
                for ci in range(2):
                    ch = 2 * su + ci
                    bnd = 32 * (ch % 4)
                    nc.tensor.matmul(
                        st[:, 512 * ci:512 * (ci + 1)],
                        k_sb[bnd:bnd + 32, 128 * ch:128 * (ch + 1)],
                        q_sb[bnd:bnd + 32, tsl],
                        start=True, stop=True,
                        tile_position=(bnd, 0))

            def emit_exp(u):
                tb, su = divmod(u, NSUP)
                st = st_tiles.pop(u)
                if su == 0:
                    p_tiles[tb] = wpool.tile([128, 512 * NCH], F8E5,
                                             tag="p", bufs=2,
                                             name=f"p_all_{tb}")
                p_all = p_tiles[tb]
                dst = p_all[:, 1024 * su:1024 * (su + 1)]
                if ENG_PATTERN[u % len(ENG_PATTERN)] == 'S':
                    nc.scalar.activation(dst, st[:], Exp,
                                         bias=0.0, scale=SCALE2)
                else:
                    nc.vector.tensor_scalar(dst.bitcast(I8), st[:],
                                            C1, C2, MULT, ADD)

            def emit_pv(u):
                tb, m = divmod(u, NSUP)
                p_all = p_tiles[tb]
                rhs = p_all[:, 1024 * m:1024 * (m + 1)].rearrange(
                    "p (two n) -> p two n", two=2)
                lhs = vT_sb[:, 128 * m:128 * (m + 1)].rearrange(
                    "p (two mm) -> p two mm", two=2)
                nc.tensor.matmul(pv_tile[:], lhs, rhs,
                                 start=(m == 0), stop=(m == NSUP - 1),
                                 perf_mode=DR, skip_group_check=True)

            def emit_epi_a(tb):
                # ONE fast ScalarE copy is pv's only reader (frees the
                # accumulator before the next t-block's first PV); DMA
                # re-bases the denominator rows to partitions 0-31.
                a_h = wpool.tile([64, 512], F32, tag="ah")
                nc.scalar.copy(a_h[:], pv_tile[:])
                dcp = wpool.tile([32, 512], F32, tag="dcp")
                nc.sync.dma_start(dcp[:], a_h[32:64, :])
                return a_h, dcp

            def emit_norm(tb, a_h, dcp):
                rc = wpool.tile([32, 512], F32, tag="rc")
                nc.vector.reciprocal_approx_fast(rc[:], dcp[:])
                an = wpool.tile([32, 512], BF16, tag="an")
                nc.vector.tensor_mul(an[:], a_h[0:32, :], rc[:])
                return an

            def emit_store(tb, an):
                op = pp_tile(f"pp_o{tb}")
                nc.tensor.matmul(op[:], wpT_sb, an[:],
                                 start=True, stop=True)
                o_sb = wpool.tile([128, 512], BF16, tag="o")
                nc.scalar.copy(o_sb[:], op[:])
                nc.sync.dma_start(
                    out_t[:, 512 * tb:512 * (tb + 1)], o_sb[:])

            NU = NTB * NSUP  # 128
            for u in range(NU + 8):
                # staggered prologue (ahead of the QK that needs it)
                if 1 <= u <= 7:
                    emit_proj(k_sb, wk_sb, bk_sb, u, f"pp_k{u}",
                              'S' if u % 2 else 'D')
                    emit_v_group(u)
                if u % NSUP == 8 and u < NU - NSUP:
                    j = u // NSUP + 1
                    emit_proj(q_sb, wq_sb, bq_sb, j, f"pp_q{j}",
                              'D' if j % 2 else 'S')
                if u < NU:
                    emit_qk(u)
                if 1 <= u < NU + 1:
                    uu = u - 1
                    emit_exp(uu)
                    emit_pv(uu)
                    tb, su = divmod(uu, NSUP)
                    if su == NSUP - 1:
                        # t-block done: free pv NOW, then stagger the
                        # rest of the epilogue
                        a_h, dcp = emit_epi_a(tb)

                        def _norm(tb=tb, a_h=a_h, dcp=dcp, u0=u):
                            an = emit_norm(tb, a_h, dcp)
                            defer(u0 + 5, lambda: emit_store(tb, an))
                        defer(u + 3, _norm)
                for fn in deferred.pop(u, ()):
                    fn()
            while deferred:
                for fn in deferred.pop(min(deferred)):
                    fn()

    nc.compile()
    return nc


def _get_nc():
    if "nc" not in _cache:
        _cache["nc"] = _build_nc()
    return _cache["nc"]


def _make_in_maps(x_, w_qkv, b_qkv, w_proj):
    bf16 = ml_dtypes.bfloat16
    in_maps = []
    for core in range(N_CORES):
        b, g = divmod(core, NH)
        wq = w_qkv[96 * g:96 * g + 32]
        wk = w_qkv[96 * g + 32:96 * g + 64]
        wv = w_qkv[96 * g + 64:96 * g + 96]
        wpT = w_proj[:, 32 * g:32 * (g + 1)].T
        wpack = np.zeros((128, 416), np.float32)
        wpack[:, 0:128] = np.tile(wq, (4, 1)).T
        wpack[:, 128:256] = np.tile(wk, (4, 1)).T
        wpack[:, 256:288] = wv.T
        wpack[0:32, 288:416] = wpT
        bpack = np.stack(
            [np.tile(b_qkv[96 * g:96 * g + 32], 4),
             np.tile(b_qkv[96 * g + 32:96 * g + 64], 4)],
            axis=1)
        in_maps.append({
            "x": x_[b].astype(bf16),
            "wpack": np.ascontiguousarray(wpack).astype(bf16),
            "bpack": np.ascontiguousarray(bpack.astype(np.float32)),
        })
    return in_maps


def _run(x, w_qkv, b_qkv, w_proj, b_proj, trace=False):
    from concourse.bass_utils import run_bass_kernel_spmd

    x_ = np.ascontiguousarray(np.asarray(x, np.float32).reshape(B, C, T))
    w_qkv = np.asarray(w_qkv, np.float32)
    b_qkv = np.asarray(b_qkv, np.float32)
    w_proj = np.asarray(w_proj, np.float32)
    b_proj = np.asarray(b_proj, np.float32)
    nc = _get_nc()

    in_maps = _make_in_maps(x_, w_qkv, b_qkv, w_proj)
    res = run_bass_kernel_spmd(nc, in_maps, core_ids=list(range(N_CORES)),
                               trace=trace)
    out = np.empty((B, C, T), np.float32)
    for b in range(B):
        acc = x_[b] + b_proj[:, None]
        for g in range(NH):
            wp = w_proj[:, 32 * g:32 * (g + 1)]
            bv = b_qkv[96 * g + 64:96 * g + 96]
            acc = acc + res.results[NH * b + g]["out"].astype(np.float32) \
                + (wp @ bv)[:, None]
        out[b] = acc
    return out.reshape(B, C, Hh, Ww), res


def kernel(x, w_qkv, b_qkv, w_proj, b_proj):
    out, _ = _run(x, w_qkv, b_qkv, w_proj, b_proj, trace=False)
    return out.astype(np.asarray(x).dtype)


# revision 14
# speedup vs baseline: 1.0622x; 1.0622x over previous
"""Trainium2 Bass kernel for nn_Attention2d (sparse_attention) — v5.

Math (per reference):
  x: (2, 128, 64, 64); T = 4096 tokens; 4 heads x 32 channels.
  qkv 1x1-conv -> per-head attention over T -> 1x1-conv out proj -> residual.

Sharding: one (batch, head) pair per core (8 cores). Each core computes its
head's attention fully on-chip and returns the per-head partial of the
output projection (bf16); the host sums the 4 head partials per batch and
adds the residual + biases (exact, fp32).

v5 design (vs v3 baseline at ~163us):
  - QK scores: bf16 matmuls (K=32 contraction, 4-band replicated q/k with
    tile-position rotation) — 1 col/cycle is the PE floor here.
  - PV: fp8 DoubleRow matmuls with the pair dim = s-chunk parity: each
    matmul contracts 256 s-tokens ([128 parts, 2 chunks, 512] plane-split
    layout), HALVING the PV column count vs bf16. vT (e4m3) carries
    32 v rows + 32 ones rows per plane so the softmax denominator
    accumulates in the same matmul.
  - softmax exp: p is e5m2 with global factor 1. ScalarE supers use
    activation-Exp directly to e5m2; VectorE supers use a Schraudolph
    bit-trick (one tensor_scalar mult+add writing int8 bits that ARE the
    e5m2 of 2^(y*log2e)). Supers (2 s-chunks = 1024 fp32 PSUM cols,
    pair-aligned) alternate between the engines ~9:7.
  - epilogue per t-block: ONE ScalarE copy drains the PV accumulator
    (frees the PSUM bank before the next t-block's first PV), a DMA
    re-bases the denominator rows to partitions 0-31, DVE reciprocal +
    multiply normalize, bf16 out-projection, ScalarE copy, bf16 DMA out
    (host accumulates partials in fp32).
  - x arrives in 8 chunked DMAs so the first projection starts early.
"""

import numpy as np
import ml_dtypes

B, C, Hh, Ww = 2, 128, 64, 64
T = Hh * Ww          # 4096
NH, CH = 4, 32
N_CORES = 8
NCH = 32             # s-chunks (of 128 tokens) per t-block
NTB = 8              # t-blocks (of 512 tokens)
NSUP = NCH // 2      # 16 supers (2 chunks each) per t-block

SCALE2 = float(1.0 / np.sqrt(CH))
# e5m2 Schraudolph: bits = round(raw*C1 + C2); int8 bits viewed as e5m2
# == 2^((bits-60)/4) ~= exp(raw*SCALE2) with global factor 1.
C1 = SCALE2 * float(np.log2(np.e)) * 4.0
C2 = 60.0

# engine for each super (S=scalar-activation, D=vector-schraudolph), 9:7
ENG_PATTERN = ['S', 'D', 'S', 'D', 'S', 'D', 'S', 'S', 'D', 'S', 'D',
               'S', 'D', 'S', 'S', 'D']

_cache = {}


def _build_nc():
    import concourse.tile as tile
    from concourse import bacc, mybir

    BF16 = mybir.dt.bfloat16
    F32 = mybir.dt.float32
    I8 = mybir.dt.int8
    F8E4 = mybir.dt.float8e4
    F8E5 = mybir.dt.float8e5
    Exp = mybir.ActivationFunctionType.Exp
    Ident = mybir.ActivationFunctionType.Identity
    MULT = mybir.AluOpType.mult
    ADD = mybir.AluOpType.add
    DR = mybir.MatmulPerfMode.DoubleRow

    nc = bacc.Bacc("TRN2", target_bir_lowering=False, debug=False,
                   num_devices=N_CORES)

    x_in = nc.dram_tensor("x", [128, T], BF16, kind="ExternalInput")
    # packed weights: wqT(4-rep) | wkT(4-rep) | wvT | wpT  ([128, 416])
    w_in = nc.dram_tensor("wpack", [128, 416], BF16, kind="ExternalInput")
    # per-partition biases (4-replicated): bq | bk
    b_in = nc.dram_tensor("bpack", [128, 2], F32, kind="ExternalInput")
    out_t = nc.dram_tensor("out", [128, T], BF16, kind="ExternalOutput")

    with tile.TileContext(nc) as tc:
        with (
            tc.tile_pool(name="const", bufs=1) as cpool,
            tc.tile_pool(name="work", bufs=2) as wpool,
            tc.tile_pool(name="psum", bufs=1, space="PSUM") as pspool,
        ):
            # ---- input DMAs: x in 8 chunks, spread across four engine
            # DMA queues so the transfers overlap ----
            x_c = []
            dma_engs = [nc.sync, nc.gpsimd, nc.scalar]
            for j in range(NTB):
                xt = cpool.tile([128, 512], BF16, tag=f"x_{j}",
                                name=f"x_{j}")
                dma_engs[j % 3].dma_start(xt[:],
                                          x_in[:, j * 512:(j + 1) * 512])
                x_c.append(xt)
            w_sb = cpool.tile([128, 416], BF16)
            nc.sync.dma_start(w_sb[:], w_in[:])
            b_sb = cpool.tile([128, 2], F32)
            nc.sync.dma_start(b_sb[:], b_in[:])
            wq_sb = w_sb[:, 0:128]
            wk_sb = w_sb[:, 128:256]
            wv_sb = w_sb[:, 256:288]
            wpT_sb = w_sb[0:32, 288:416]
            bq_sb = b_sb[:, 0:1]
            bk_sb = b_sb[:, 1:2]

            q_sb = cpool.tile([128, T], BF16)
            k_sb = cpool.tile([128, T], BF16)
            # vT pair-blocks: block m: [2 planes x (32 v + 32 ones)] e4m3
            vT_sb = cpool.tile([128, 64 * NCH], F8E4)
            nc.gpsimd.memset(vT_sb[:], 1.0)

            # ---- helpers ----
            def pp_tile(nm):
                return pspool.tile([128, 512], F32, tag="pp", bufs=1,
                                   name=nm)

            def emit_proj(dst, wsb, bsb, j, nm, eng):
                """one 512-col t-chunk of the q/k projection (4-band
                replicated), bias added in the PSUM->bf16 copy."""
                ps = pp_tile(nm)
                nc.tensor.matmul(ps[:], wsb, x_c[j][:],
                                 start=True, stop=True)
                dstsl = dst[:, 512 * j:512 * (j + 1)]
                if eng == 'S':
                    nc.scalar.activation(dstsl, ps[:], Ident, bias=bsb,
                                         scale=1.0)
                else:
                    nc.vector.tensor_scalar_add(dstsl, ps[:], bsb)

            def emit_v_group(g):
                """v^T projection for s-chunks 4g..4g+3 (x chunk g) into
                vT pair-blocks 2g, 2g+1 (plane-split, e4m3)."""
                ps = pp_tile(f"pp_v{g}")
                for cc in range(4):
                    nc.tensor.matmul(
                        ps[:, 32 * cc:32 * (cc + 1)],
                        x_c[g][:, 128 * cc:128 * (cc + 1)],
                        wv_sb, start=True, stop=True)
                src = ps[:, 0:128].rearrange("p (c m) -> p c m", m=32)
                dst = vT_sb[:, 256 * g:256 * (g + 1)].rearrange(
                    "p (c m) -> p c m", m=64)[:, :, 0:32]
                nc.vector.tensor_copy(dst, src)

            # ---- prologue: minimum for t-block 0 ----
            emit_proj(k_sb, wk_sb, bk_sb, 0, "pp_k0", 'S')
            emit_proj(q_sb, wq_sb, bq_sb, 0, "pp_q0", 'D')
            emit_v_group(0)

            # ---- attention, software-pipelined over supers ----
            st_tiles = {}
            p_tiles = {}
            pv_tile = pspool.tile([64, 512], F32, tag="pv", bufs=1,
                                  name="pv")
            deferred = {}

            def defer(u, fn):
                deferred.setdefault(u, []).append(fn)

            def emit_qk(u):
                tb, su = divmod(u, NSUP)
                st = pspool.tile([128, 1024], F32, tag="st", bufs=3,
                                 name=f"st_{u}")
                st_tiles[u] = st
                tsl = slice(512 * tb, 512 * (tb + 1))
                for ci in range(2):
                    ch = 2 * su + ci
                    bnd = 32 * (ch % 4)
                    nc.tensor.matmul(
                        st[:, 512 * ci:512 * (ci + 1)],
                        k_sb[bnd:bnd + 32, 128 * ch:128 * (ch + 1)],
                        q_sb[bnd:bnd + 32, tsl],
                        start=True, stop=True,
                        tile_position=(bnd, 0))

            def emit_exp(u):
                tb, su = divmod(u, NSUP)
                st = st_tiles.pop(u)
                if su == 0:
                    p_tiles[tb] = wpool.tile([128, 512 * NCH], F8E5,
                                             tag="p", bufs=2,
                                             name=f"p_all_{tb}")
                p_all = p_tiles[tb]
                dst = p_all[:, 1024 * su:1024 * (su + 1)]
                if ENG_PATTERN[u % len(ENG_PATTERN)] == 'S':
                    nc.scalar.activation(dst, st[:], Exp,
                                         bias=0.0, scale=SCALE2)
                else:
                    nc.vector.tensor_scalar(dst.bitcast(I8), st[:],
                                            C1, C2, MULT, ADD)

            def emit_pv(u):
                tb, m = divmod(u, NSUP)
                p_all = p_tiles[tb]
                rhs = p_all[:, 1024 * m:1024 * (m + 1)].rearrange(
                    "p (two n) -> p two n", two=2)
                lhs = vT_sb[:, 128 * m:128 * (m + 1)].rearrange(
                    "p (two mm) -> p two mm", two=2)
                nc.tensor.matmul(pv_tile[:], lhs, rhs,
                                 start=(m == 0), stop=(m == NSUP - 1),
                                 perf_mode=DR, skip_group_check=True)

            def emit_epi_a(tb):
                # ONE fast ScalarE copy is pv's only reader (frees the
                # accumulator before the next t-block's first PV); DMA
                # re-bases the denominator rows to partitions 0-31.
                a_h = wpool.tile([64, 512], F32, tag="ah")
                nc.scalar.copy(a_h[:], pv_tile[:])
                dcp = wpool.tile([32, 512], F32, tag="dcp")
                nc.sync.dma_start(dcp[:], a_h[32:64, :])
                return a_h, dcp

            def emit_norm(tb, a_h, dcp):
                rc = wpool.tile([32, 512], F32, tag="rc")
                nc.vector.reciprocal_approx_fast(rc[:], dcp[:])
                an = wpool.tile([32, 512], BF16, tag="an")
                nc.vector.tensor_mul(an[:], a_h[0:32, :], rc[:])
                return an

            def emit_store(tb, an):
                op = pp_tile(f"pp_o{tb}")
                nc.tensor.matmul(op[:], wpT_sb, an[:],
                                 start=True, stop=True)
                o_sb = wpool.tile([128, 512], BF16, tag="o")
                nc.scalar.copy(o_sb[:], op[:])
                nc.sync.dma_start(
                    out_t[:, 512 * tb:512 * (tb + 1)], o_sb[:])

            # PV lags the exp by 2 extra supers so the in-order PE queue
            # never gates one engine's exp behind the other's: at slot u
            # the PE runs QK(u) and PV(u-3), whose exp finished ~2 slots
            # ago, while exp(u-1)/exp(u-2) run concurrently on the two
            # exp engines.
            NU = NTB * NSUP  # 128
            LAG = 3
            for u in range(NU + LAG + 8):
                # staggered prologue (just ahead of the QK that needs it)
                if u % 2 == 1 and 1 <= (u + 1) // 2 <= 7:
                    j = (u + 1) // 2
                    emit_proj(k_sb, wk_sb, bk_sb, j, f"pp_k{j}",
                              'S' if j % 2 else 'D')
                if u % 2 == 0 and 1 <= u // 2 <= 7:
                    emit_v_group(u // 2)
                if u % NSUP == 8 and u < NU - NSUP:
                    j = u // NSUP + 1
                    emit_proj(q_sb, wq_sb, bq_sb, j, f"pp_q{j}",
                              'D' if j % 2 else 'S')
                if u < NU:
                    emit_qk(u)
                if 1 <= u < NU + 1:
                    emit_exp(u - 1)
                if LAG <= u < NU + LAG:
                    uu = u - LAG
                    emit_pv(uu)
                    tb, su = divmod(uu, NSUP)
                    if su == NSUP - 1:
                        # t-block done: free pv NOW, then stagger the
                        # rest of the epilogue
                        p_tiles.pop(tb - 1, None)
                        a_h, dcp = emit_epi_a(tb)

                        def _norm(tb=tb, a_h=a_h, dcp=dcp, u0=u):
                            an = emit_norm(tb, a_h, dcp)
                            defer(u0 + 5, lambda: emit_store(tb, an))
                        defer(u + 3, _norm)
                for fn in deferred.pop(u, ()):
                    fn()
            while deferred:
                for fn in deferred.pop(min(deferred)):
                    fn()

    nc.compile()
    return nc


def _get_nc():
    if "nc" not in _cache:
        _cache["nc"] = _build_nc()
    return _cache["nc"]


def _make_in_maps(x_, w_qkv, b_qkv, w_proj):
    bf16 = ml_dtypes.bfloat16
    in_maps = []
    for core in range(N_CORES):
        b, g = divmod(core, NH)
        wq = w_qkv[96 * g:96 * g + 32]
        wk = w_qkv[96 * g + 32:96 * g + 64]
        wv = w_qkv[96 * g + 64:96 * g + 96]
        wpT = w_proj[:, 32 * g:32 * (g + 1)].T
        wpack = np.zeros((128, 416), np.float32)
        wpack[:, 0:128] = np.tile(wq, (4, 1)).T
        wpack[:, 128:256] = np.tile(wk, (4, 1)).T
        wpack[:, 256:288] = wv.T
        wpack[0:32, 288:416] = wpT
        bpack = np.stack(
            [np.tile(b_qkv[96 * g:96 * g + 32], 4),
             np.tile(b_qkv[96 * g + 32:96 * g + 64], 4)],
            axis=1)
        in_maps.append({
            "x": x_[b].astype(bf16),
            "wpack": np.ascontiguousarray(wpack).astype(bf16),
            "bpack": np.ascontiguousarray(bpack.astype(np.float32)),
        })
    return in_maps


def _run(x, w_qkv, b_qkv, w_proj, b_proj, trace=False):
    from concourse.bass_utils import run_bass_kernel_spmd

    x_ = np.ascontiguousarray(np.asarray(x, np.float32).reshape(B, C, T))
    w_qkv = np.asarray(w_qkv, np.float32)
    b_qkv = np.asarray(b_qkv, np.float32)
    w_proj = np.asarray(w_proj, np.float32)
    b_proj = np.asarray(b_proj, np.float32)
    nc = _get_nc()

    in_maps = _make_in_maps(x_, w_qkv, b_qkv, w_proj)
    res = run_bass_kernel_spmd(nc, in_maps, core_ids=list(range(N_CORES)),
                               trace=trace)
    out = np.empty((B, C, T), np.float32)
    for b in range(B):
        acc = x_[b] + b_proj[:, None]
        for g in range(NH):
            wp = w_proj[:, 32 * g:32 * (g + 1)]
            bv = b_qkv[96 * g + 64:96 * g + 96]
            acc = acc + res.results[NH * b + g]["out"].astype(np.float32) \
                + (wp @ bv)[:, None]
        out[b] = acc
    return out.reshape(B, C, Hh, Ww), res


def kernel(x, w_qkv, b_qkv, w_proj, b_proj):
    out, _ = _run(x, w_qkv, b_qkv, w_proj, b_proj, trace=False)
    return out.astype(np.asarray(x).dtype)
